# revision 24
# baseline (speedup 1.0000x reference)
"""Bass/Trainium2 kernel for nn_CausalSelfAttention_15504831939088.

Multi-head attention with a key-length mask, B=2 S=2048 D=1024 H=16 DH=64,
run on 8 NeuronCores: data parallel over batch (2) x tensor parallel over
head groups (4 heads per core).  Each core computes, for its (batch b,
head-group g):

    xT       = x[b]^T                       (PE transposes)
    Q^T,K^T  = (x[b] @ Wq_cols + bq)^T      [256, 2048]  dh on partitions
    V        = x[b] @ Wv_cols + bv          [2048, 256]  s on partitions
    S^T      = K_h @ Q_h^T                  per head, per key-tile
    P^T      = exp(S^T * 0.125 + mask_k)    mask folded into ACT bias
    ctx^T    = [1|V_h]^T-style PV with a leading ones column producing the
               softmax denominator on partition 0 "for free"
    out_par  = (ctx^T/denom)^T-packed @ Wo_rows + bo_eff   [2048, 1024]

The host sums the 4 head-group partials per batch (the Megatron-style
row-parallel all-reduce done at unshard time).

Scores are bounded (|s/8| < ~10 for these input stats), so softmax skips the
max-subtraction pass entirely; masked keys get bias -1e30 -> exp == 0.
"""

import numpy as np

B, S, D, H = 2, 2048, 1024, 16
DH = D // H  # 64
HPC = 4      # heads per core
DHC = HPC * DH  # 256 cols per core
NST = S // 128  # 16 s-tiles
NKT = D // 128  # 8 contraction tiles over D
NQH = 2      # q halves (1024 each)
QH = S // NQH

_CACHE = {}


def _build(loop=1):
    """Build the SPMD Bass program + a reusable jitted runner. Cached.

    loop > 1 emits the whole per-core body `loop` times inside one NEFF --
    used only for timing (wall-clock slope over loop count isolates the
    on-device execution time from the per-call dispatch/transfer overhead).
    """
    import os as _os
    _key = (loop, _os.environ.get("BASS_PTBUFS", "4"),
            _os.environ.get("BASS_TRBUFS", "2"),
            _os.environ.get("BASS_PRBUFS", "4"),
            _os.environ.get("BASS_PHASES", "all"),
            _os.environ.get("BASS_EXPVIA", "psum"))
    if ("run", _key) in _CACHE:
        return _CACHE[("run", _key)]

    import jax
    import concourse.bass as bass
    import concourse.mybir as mybir
    import concourse.tile as tile
    from concourse import bacc, bass2jax
    from concourse.bass2jax import _bass_exec_p, partition_id_tensor
    from concourse.masks import make_identity
    from jax.sharding import Mesh, PartitionSpec
    from jax.experimental.shard_map import shard_map

    f32 = mybir.dt.float32
    f32r = mybir.dt.float32r

    nc = bacc.Bacc("TRN2", target_bir_lowering=False, debug=False, num_devices=8)

    xb = nc.dram_tensor("xb", [S, D], f32, kind="ExternalInput").ap()
    wq = nc.dram_tensor("wq", [D, DHC], f32, kind="ExternalInput").ap()
    wk = nc.dram_tensor("wk", [D, DHC], f32, kind="ExternalInput").ap()
    wv = nc.dram_tensor("wv", [D, DHC], f32, kind="ExternalInput").ap()
    wo = nc.dram_tensor("wo", [DHC, D], f32, kind="ExternalInput").ap()
    bq = nc.dram_tensor("bq", [DHC], f32, kind="ExternalInput").ap()
    bk = nc.dram_tensor("bk", [DHC], f32, kind="ExternalInput").ap()
    bv = nc.dram_tensor("bv", [DHC], f32, kind="ExternalInput").ap()
    bo = nc.dram_tensor("bo", [D], f32, kind="ExternalInput").ap()
    msk = nc.dram_tensor("msk", [128, NST], f32, kind="ExternalInput").ap()
    y = nc.dram_tensor("out", [S, D], f32, kind="ExternalOutput").ap()
    import os
    DBG = bool(os.environ.get("BASSDBG"))
    PHASES = os.environ.get("BASS_PHASES", "all")
    EXPVIA = os.environ.get("BASS_EXPVIA", "psum")
    PTBUFS = int(os.environ.get("BASS_PTBUFS", "4"))
    TRBUFS = int(os.environ.get("BASS_TRBUFS", "2"))
    PRBUFS = int(os.environ.get("BASS_PRBUFS", "4"))
    if DBG:
        dqT = nc.dram_tensor("dqT", [2, 128, S], f32, kind="ExternalOutput").ap()
        dkT = nc.dram_tensor("dkT", [2, 128, S], f32, kind="ExternalOutput").ap()
        dva = nc.dram_tensor("dva", [NST, 128, HPC * (DH + 1)], f32,
                             kind="ExternalOutput").ap()
        dctxn = nc.dram_tensor("dctxn", [2, 128, S], f32,
                               kind="ExternalOutput").ap()
        dpt = nc.dram_tensor("dpt", [128, QH], f32, kind="ExternalOutput").ap()
        drc = nc.dram_tensor("drc", [1, QH], f32, kind="ExternalOutput").ap()

    def r(ap):
        return ap

    from contextlib import ExitStack

    def emit_body(tc):
        with ExitStack() as ctx:
            persist = ctx.enter_context(tc.tile_pool(name="persist", bufs=1))
            # -------- tiny constants --------
            ident = persist.tile([128, 128], f32)
            make_identity(nc, ident)
            mask_sb = persist.tile([128, NST], f32)
            nc.sync.dma_start(out=mask_sb, in_=msk)

            qT = [persist.tile([128, S], f32r, tag=f"qT{j}", name=f"qT{j}")
                  for j in range(2)]
            kT = [persist.tile([128, S], f32r, tag=f"kT{j}", name=f"kT{j}")
                  for j in range(2)]
            # V augmented with a trailing ones column per head: [128, 4, 65]
            v_aug = [persist.tile([128, HPC * (DH + 1)], f32r, tag=f"va{s}",
                                  name=f"va{s}") for s in range(NST)]

            # -------- phase 1+2: transpose x, projections --------
            with ExitStack() as s1:
                xs_pool = s1.enter_context(tc.tile_pool(name="xstage", bufs=4))
                xt_pool = s1.enter_context(tc.tile_pool(name="xT", bufs=1))
                wst_pool = s1.enter_context(tc.tile_pool(name="wstage",
                                                         bufs=4))
                ps_tr = s1.enter_context(
                    tc.tile_pool(name="ps_tr", bufs=TRBUFS, space="PSUM"))
                ps_pr = s1.enter_context(
                    tc.tile_pool(name="ps_pr", bufs=PRBUFS, space="PSUM"))
                ps_pv = s1.enter_context(
                    tc.tile_pool(name="ps_pv", bufs=2, space="PSUM"))

                # x loads + PE transposes first (weights DMA later so the
                # first transpose isn't queued behind 4MB of weights)
                xT = xt_pool.tile([128, NKT, S], f32r, name="xT")
                for st in range(NST):
                    xs = xs_pool.tile([128, D], f32)
                    nc.sync.dma_start(out=xs,
                                      in_=xb[st * 128:(st + 1) * 128, :])
                    for k4 in range(NKT // 4):
                        pt = ps_tr.tile([128, 4, 128], f32)
                        for k in range(4):
                            nc.tensor.transpose(
                                pt[:, k, :],
                                xs[:, (k4 * 4 + k) * 128:
                                   (k4 * 4 + k + 1) * 128], ident)
                        nc.vector.tensor_copy(
                            out=xT[:, k4 * 4:(k4 + 1) * 4,
                                   st * 128:(st + 1) * 128],
                            in_=pt)

                # -------- weight / bias loads (f32r via ScalarE copies) ----
                wq_t = []
                wk_t = []
                wv_t = []
                for k in range(NKT):
                    for nm, src_ap, lst in (("wv", wv, wv_t), ("wq", wq, wq_t),
                                            ("wk", wk, wk_t)):
                        stg = wst_pool.tile([128, DHC], f32, tag="wst",
                                            name=f"{nm}s{k}")
                        nc.sync.dma_start(
                            out=stg, in_=src_ap[k * 128:(k + 1) * 128, :])
                        t = persist.tile([128, DHC], f32r, tag=f"{nm}{k}",
                                         name=f"{nm}{k}")
                        nc.scalar.copy(out=t, in_=stg)
                        lst.append(t)
                wo_t = []
                for j in range(2):
                    stg = wst_pool.tile([128, D], f32, tag="wost",
                                        name=f"wos{j}")
                    nc.sync.dma_start(
                        out=stg, in_=wo[j * 128:(j + 1) * 128, :])
                    t = persist.tile([128, D], f32r, tag=f"wo{j}",
                                     name=f"wo{j}")
                    nc.scalar.copy(out=t, in_=stg)
                    wo_t.append(t)
                bq_sb = persist.tile([128, 2], f32)
                bk_sb = persist.tile([128, 2], f32)
                for j in range(2):
                    nc.sync.dma_start(out=bq_sb[:, j:j + 1],
                                      in_=bq[j * 128:(j + 1) * 128][:, None])
                    nc.sync.dma_start(out=bk_sb[:, j:j + 1],
                                      in_=bk[j * 128:(j + 1) * 128][:, None])
                ones4 = persist.tile([128, HPC], f32)
                nc.vector.memset(ones4, 1.0)
                bv_bc = persist.tile([128, DHC], f32)
                nc.sync.dma_start(
                    out=bv_bc,
                    in_=bass.AP(tensor=bv.tensor, offset=bv.offset,
                                ap=[[0, 128], [1, DHC]]))
                bo_bc = persist.tile([128, D], f32)
                nc.sync.dma_start(
                    out=bo_bc,
                    in_=bass.AP(tensor=bo.tensor, offset=bo.offset,
                                ap=[[0, 128], [1, D]]))

                # V natural [s 128, dh' 256] + bias, interleaved into v_aug
                # (emitted before Q/K so the attention PV dependency clears
                # early; Q/K for the second head pair can fill PE slack
                # under the ACT-bound attention phase)
                for st in range(NST):
                    pv = ps_pv.tile([128, DHC], f32, tag="pv")
                    for k in range(NKT):
                        nc.tensor.matmul(
                            pv, xT[:, k, st * 128:(st + 1) * 128],
                            wv_t[k], start=(k == 0), stop=(k == NKT - 1))
                    va = v_aug[st].rearrange("p (h c) -> p h c", c=DH + 1)
                    nc.vector.tensor_copy(out=va[:, :, DH:DH + 1],
                                          in_=ones4[:, :, None])
                    nc.vector.tensor_add(
                        out=va[:, :, 0:DH],
                        in0=pv.rearrange("p (h c) -> p h c", c=DH),
                        in1=bv_bc.rearrange("p (h c) -> p h c", c=DH))

                # Q^T, K^T : [dh' 128, s 512] chunks, contract D
                for j in range(2):
                    for sc in range(4):
                        pq = ps_pr.tile([128, 512], f32, tag="pp")
                        pk = ps_pr.tile([128, 512], f32, tag="pp")
                        for k in range(NKT):
                            nc.tensor.matmul(
                                pq, wq_t[k][:, j * 128:(j + 1) * 128],
                                xT[:, k, sc * 512:(sc + 1) * 512],
                                start=(k == 0), stop=(k == NKT - 1))
                        for k in range(NKT):
                            nc.tensor.matmul(
                                pk, wk_t[k][:, j * 128:(j + 1) * 128],
                                xT[:, k, sc * 512:(sc + 1) * 512],
                                start=(k == 0), stop=(k == NKT - 1))
                        nc.vector.tensor_scalar_add(
                            out=qT[j][:, sc * 512:(sc + 1) * 512], in0=pq,
                            scalar1=bq_sb[:, j:j + 1])
                        nc.vector.tensor_scalar_add(
                            out=kT[j][:, sc * 512:(sc + 1) * 512], in0=pk,
                            scalar1=bk_sb[:, j:j + 1])

            if DBG:
                for j in range(2):
                    nc.sync.dma_start(out=dqT[j], in_=qT[j].bitcast(f32))
                    nc.sync.dma_start(out=dkT[j], in_=kT[j].bitcast(f32))
                for st in range(NST):
                    nc.sync.dma_start(out=dva[st], in_=v_aug[st].bitcast(f32))

            # -------- phase 3: attention --------
            if PHASES == "front":
                return
            ctxn = [persist.tile([128, S], f32r, tag=f"ctxn{j}",
                                 name=f"ctxn{j}") for j in range(2)]
            with ExitStack() as s2:
                pt_pool = s2.enter_context(tc.tile_pool(name="pT", bufs=PTBUFS))
                rc_pool = s2.enter_context(tc.tile_pool(name="recip", bufs=2))
                ps_sc = s2.enter_context(
                    tc.tile_pool(name="ps_sc", bufs=2, space="PSUM"))
                ps_cx = s2.enter_context(
                    tc.tile_pool(name="ps_cx", bufs=2, space="PSUM"))

                for h in range(HPC):
                    j = h // 2
                    p0 = (h % 2) * 64
                    for qh in range(NQH):
                        cx = ps_cx.tile([DH + 1, QH], f32)

                        def scores(kt):
                            sc = ps_sc.tile([128, QH], f32, name="sc")
                            for c in range(QH // 512):
                                nc.tensor.matmul(
                                    sc[:, c * 512:(c + 1) * 512],
                                    kT[j][p0:p0 + 64,
                                          kt * 128:(kt + 1) * 128],
                                    qT[j][p0:p0 + 64,
                                          qh * QH + c * 512:
                                          qh * QH + (c + 1) * 512])
                            return sc

                        # software pipeline: emit scores(kt+1) before PV(kt)
                        # so the in-order PE stream fills the exp(kt) wait.
                        sc_cur = scores(0)
                        for kt in range(NST):
                            pt = pt_pool.tile([128, QH], f32r)
                            nc.scalar.activation(
                                out=pt, in_=sc_cur,
                                func=mybir.ActivationFunctionType.Exp,
                                bias=mask_sb[:, kt:kt + 1], scale=0.125)
                            if DBG and h == 0 and qh == 0 and kt == 0:
                                nc.sync.dma_start(out=dpt,
                                                  in_=pt.bitcast(f32))
                            if kt + 1 < NST:
                                sc_cur = scores(kt + 1)
                            for c in range(QH // 512):
                                nc.tensor.matmul(
                                    cx[:, c * 512:(c + 1) * 512],
                                    v_aug[kt][:, h * (DH + 1):
                                              (h + 1) * (DH + 1)],
                                    pt[:, c * 512:(c + 1) * 512],
                                    start=(kt == 0), stop=(kt == NST - 1))
                        rc = rc_pool.tile([65, QH], f32, tag="rc")
                        nc.vector.reciprocal(
                            out=rc[64:65, :], in_=cx[64:65, :])
                        rc0 = rc_pool.tile([1, QH], f32, tag="rc0")
                        nc.sync.dma_start(out=rc0, in_=rc[64:65, :])
                        bc64 = rc_pool.tile([64, QH], f32, tag="bc64")
                        nc.gpsimd.partition_broadcast(bc64, rc0, channels=64)
                        st64 = rc_pool.tile([64, QH], f32r, tag="st64")
                        nc.vector.tensor_mul(
                            out=st64, in0=cx[0:64, :], in1=bc64)
                        nc.sync.dma_start(
                            out=ctxn[j][p0:p0 + 64, qh * QH:(qh + 1) * QH],
                            in_=st64)
                        if DBG and h == 0 and qh == 0:
                            nc.sync.dma_start(out=drc, in_=rc[64:65, :])

            if DBG:
                for j in range(2):
                    nc.sync.dma_start(out=dctxn[j], in_=ctxn[j].bitcast(f32))

            # -------- phase 4: output projection --------
            if PHASES == "attn":
                return
            with ExitStack() as s3:
                out_pool = s3.enter_context(tc.tile_pool(name="osb", bufs=3))
                ps_o = s3.enter_context(
                    tc.tile_pool(name="ps_o", bufs=2, space="PSUM"))
                for st in range(NST):
                    ot = out_pool.tile([128, D], f32)
                    for dc in range(2):
                        po = ps_o.tile([128, 512], f32)
                        for j in range(2):
                            nc.tensor.matmul(
                                po, ctxn[j][:, st * 128:(st + 1) * 128],
                                wo_t[j][:, dc * 512:(dc + 1) * 512],
                                start=(j == 0), stop=(j == 1))
                        nc.vector.tensor_add(
                            out=ot[:, dc * 512:(dc + 1) * 512], in0=po,
                            in1=bo_bc[:, dc * 512:(dc + 1) * 512])
                    nc.sync.dma_start(out=y[st * 128:(st + 1) * 128, :],
                                      in_=ot)

    with tile.TileContext(nc) as tc:
        for _ in range(loop):
            emit_body(tc)

    nc.compile()

    # ---- reusable PJRT runner (mirrors bass2jax.run_bass_via_pjrt) ----
    bass2jax.install_neuronx_cc_hook()
    partition_name = (nc.partition_id_tensor.name
                      if nc.partition_id_tensor else None)
    in_names, out_names, out_avals, zero_outs = [], [], [], []
    for alloc in nc.m.functions[0].allocations:
        if not isinstance(alloc, mybir.MemoryLocationSet):
            continue
        name = alloc.memorylocations[0].name
        if alloc.kind == "ExternalInput":
            if name != partition_name:
                in_names.append(name)
        elif alloc.kind == "ExternalOutput":
            out_names.append(name)
            shape = tuple(alloc.tensor_shape)
            dtype = mybir.dt.np(alloc.dtype)
            out_avals.append(jax.core.ShapedArray(shape, dtype))
            zero_outs.append(np.zeros(shape, dtype))
    n_params = len(in_names)
    in_names_all = in_names + out_names + (
        [partition_name] if partition_name else [])

    def _body(*args):
        operands = list(args)
        if partition_name is not None:
            operands.append(partition_id_tensor())
        return tuple(_bass_exec_p.bind(
            *operands, out_avals=tuple(out_avals),
            in_names=tuple(in_names_all), out_names=tuple(out_names),
            lowering_input_output_aliases=(), sim_require_finite=True,
            sim_require_nnan=True, nc=nc))

    devices = jax.devices()[:8]
    mesh = Mesh(np.asarray(devices), ("core",))
    nio = n_params + len(out_names)
    sharded = jax.jit(
        shard_map(_body, mesh=mesh, in_specs=(PartitionSpec("core"),) * nio,
                  out_specs=(PartitionSpec("core"),) * len(out_names),
                  check_rep=False),
        keep_unused=True)

    def prep(in_maps):
        concat_in = [
            np.concatenate([np.asarray(m[name]) for m in in_maps], axis=0)
            for name in in_names]
        concat_zeros = [
            np.zeros((8 * z.shape[0], *z.shape[1:]), z.dtype)
            for z in zero_outs]
        return concat_in + concat_zeros

    def run(in_maps):
        outs = sharded(*prep(in_maps))
        arr = np.asarray(outs[out_names.index("out")])
        _CACHE["last_outs"] = {n: np.asarray(outs[i])
                               for i, n in enumerate(out_names)}
        return arr.reshape(8, S, D)

    _CACHE[("run", _key)] = run
    _CACHE[("run", loop)] = run
    _CACHE[("sharded", _key)] = sharded
    _CACHE[("sharded", loop)] = sharded
    _CACHE["prep"] = prep
    _CACHE["out_index"] = out_names.index("out")
    return run


def _shard_inputs(x, valid_nums, Wq, bq, Wk, bk, Wv, bv, Wo, bo):
    in_maps = []
    x = np.asarray(x, dtype=np.float32)
    idx = np.arange(S)
    for c in range(8):
        b, g = divmod(c, 4)
        sl = slice(g * DHC, (g + 1) * DHC)
        vn = int(np.asarray(valid_nums)[b])
        mask = np.where(idx < vn, 0.0, -1e30).astype(np.float32)
        in_maps.append({
            "xb": np.ascontiguousarray(x[b]),
            "wq": np.ascontiguousarray(np.asarray(Wq, np.float32)[:, sl]),
            "wk": np.ascontiguousarray(np.asarray(Wk, np.float32)[:, sl]),
            "wv": np.ascontiguousarray(np.asarray(Wv, np.float32)[:, sl]),
            "wo": np.ascontiguousarray(np.asarray(Wo, np.float32)[sl, :]),
            "bq": np.ascontiguousarray(np.asarray(bq, np.float32)[sl]),
            "bk": np.ascontiguousarray(np.asarray(bk, np.float32)[sl]),
            "bv": np.ascontiguousarray(np.asarray(bv, np.float32)[sl]),
            "bo": (np.asarray(bo, np.float32) if g == 0
                   else np.zeros(D, np.float32)),
            "msk": np.ascontiguousarray(mask.reshape(NST, 128).T),
        })
    return in_maps


def kernel(x, valid_nums, Wq, bq, Wk, bk, Wv, bv, Wo, bo):
    run = _build()
    in_maps = _shard_inputs(x, valid_nums, Wq, bq, Wk, bk, Wv, bv, Wo, bo)
    parts = run(in_maps)  # [8, S, D]
    out = np.empty((B, S, D), dtype=np.float32)
    for b in range(B):
        out[b] = parts[4 * b] + parts[4 * b + 1] + parts[4 * b + 2] \
            + parts[4 * b + 3]
    return out
